# revision 1
# baseline (speedup 1.0000x reference)
"""Causal single-head attention (B=4, S=2048, D=1024) on 8 Trainium2 cores.

Sharding: 8 cores = (batch b, key-half kh). Core (b, kh) owns keys/values
for rows [kh*1024, kh*1024+1024) of batch b and computes partial (un-
normalized) attention for ALL queries that can see those keys. Queries are
fed "aligned": query slot r corresponds to global row kh*1024 + r, so the
causal condition is always r >= j (j = local key index) — identical on
every core, letting one SPMD program serve both key halves with a purely
compile-time block-triangular mask. Slots past the end of the sequence
(kh=1, r >= 1024) compute junk that the host discards.

Softmax uses no max-subtraction (logits are O(1) for this problem:
|score/32| < ~4), so per-core partials are just num = exp(S)·V and
l = sum(exp(S)); the host merges halves with num/den addition and one
divide. This is mathematically identical to the reference softmax.

On-chip layout (all matmul operands bf16, fp32 PSUM accumulation):
  xt  = x_core^T              [D=1024, NQ=2048]  (keys are cols 0:1024)
  QT[o,r], KT[o,j] transposed; V[j,d] natural — chosen so every matmul
  is a plain lhsT.T @ rhs with no on-chip transposes:
    KT = wkT.T @ xt,  QT = wqT.T @ xt,  V = xt.T @ wvT
    ST = KT.T @ QT    (scores transposed: partition=key, free=query)
    PT = exp(ST/32) masked to r>=j;  OT = V.T @ PT;  l = 1s.T @ PT
"""

import sys

sys.path.insert(0, "/opt/trn_rl_repo")

from contextlib import ExitStack

import ml_dtypes
import numpy as np

import concourse.bass as bass  # noqa: F401  (engine types resolve via bacc)
import concourse.mybir as mybir
import concourse.tile as tile
from concourse import bacc, bass_utils
from concourse.bass import ts

BF16 = mybir.dt.bfloat16
F32 = mybir.dt.float32

P = 128            # partitions
D = 1024           # model dim (d_in == d_out)
NQ = 2048          # query slots per core
NK = 1024          # keys per core
RC = 512           # query-chunk (matmul moving-dim) size
N_RC = NQ // RC    # 4
N_KT = NK // P     # 8 key tiles
N_IB = D // P      # 8 contraction blocks
SCALE = 1.0 / 32.0 # 1/sqrt(D)

N_CORES = 8
B, S = 4, 2048
H = S // 2


def _kept_kts(rc):
    # (kt, rc) pairs where some query in the chunk can see some key of the
    # tile: min r = rc*RC, max j = kt*P + 127; causal keeps r >= j.
    return [kt for kt in range(N_KT) if kt * P <= rc * RC + RC - 1]


def _emit(nc, tc, xt, wqt, wkt, wvt, ot, ls):
    with ExitStack() as ctx:
        sb = ctx.enter_context(tc.tile_pool(name="sb", bufs=1))
        pts = ctx.enter_context(tc.tile_pool(name="pts", bufs=2))
        outp = ctx.enter_context(tc.tile_pool(name="outp", bufs=3))
        ps = ctx.enter_context(tc.tile_pool(name="ps", bufs=4, space="PSUM"))
        psl = ctx.enter_context(tc.tile_pool(name="psl", bufs=2, space="PSUM"))

        ones = sb.tile([P, 1], BF16, tag="ones", name="ones")
        nc.vector.memset(ones, 1.0)

        # ---- input loads ----
        xt_sb = []
        for i in range(N_IB):
            t = sb.tile([P, NQ], BF16, tag=f"xt{i}", name=f"xt{i}")
            nc.sync.dma_start(out=t, in_=xt[ts(i, P), :])
            xt_sb.append(t)
        w_sb = {}
        for nm, dram in (("wk", wkt), ("wq", wqt), ("wv", wvt)):
            w_sb[nm] = []
            for i in range(N_IB):
                t = sb.tile([P, D], BF16, tag=f"{nm}{i}", name=f"{nm}{i}")
                nc.sync.dma_start(out=t, in_=dram[ts(i, P), :])
                w_sb[nm].append(t)

        # ---- projections ----
        kt_sb = [sb.tile([P, NK], BF16, tag=f"ktk{o}", name=f"ktk{o}")
                 for o in range(N_IB)]
        for o in range(N_IB):
            for jc in range(NK // RC):
                acc_kt = ps.tile([P, RC], F32, tag="mm", name="acc_kt")
                for i in range(N_IB):
                    nc.tensor.matmul(acc_kt,
                                     lhsT=w_sb["wk"][i][:, ts(o, P)],
                                     rhs=xt_sb[i][:, ts(jc, RC)],
                                     start=(i == 0), stop=(i == N_IB - 1))
                nc.vector.tensor_copy(kt_sb[o][:, ts(jc, RC)], acc_kt)

        qt_sb = [sb.tile([P, NQ], BF16, tag=f"qtq{o}", name=f"qtq{o}")
                 for o in range(N_IB)]
        for o in range(N_IB):
            for rc in range(N_RC):
                acc_qt = ps.tile([P, RC], F32, tag="mm", name="acc_qt")
                for i in range(N_IB):
                    nc.tensor.matmul(acc_qt,
                                     lhsT=w_sb["wq"][i][:, ts(o, P)],
                                     rhs=xt_sb[i][:, ts(rc, RC)],
                                     start=(i == 0), stop=(i == N_IB - 1))
                nc.vector.tensor_copy(qt_sb[o][:, ts(rc, RC)], acc_qt)

        v_sb = [sb.tile([P, D], BF16, tag=f"vj{j}", name=f"vj{j}")
                for j in range(N_KT)]
        for j in range(N_KT):
            for dc in range(D // RC):
                acc_v = ps.tile([P, RC], F32, tag="mm", name="acc_v")
                for i in range(N_IB):
                    nc.tensor.matmul(acc_v,
                                     lhsT=xt_sb[i][:, ts(j, P)],
                                     rhs=w_sb["wv"][i][:, ts(dc, RC)],
                                     start=(i == 0), stop=(i == N_IB - 1))
                nc.vector.tensor_copy(v_sb[j][:, ts(dc, RC)], acc_v)

        # ---- attention ----
        pt_tiles = {}

        def emit_st(rc):
            kts = _kept_kts(rc)
            lp = psl.tile([1, RC], F32, tag="lp", name="lp")
            for n, kt in enumerate(kts):
                acc_st = ps.tile([P, RC], F32, tag="mm", name="acc_st")
                for o in range(N_IB):
                    nc.tensor.matmul(acc_st,
                                     lhsT=kt_sb[o][:, ts(kt, P)],
                                     rhs=qt_sb[o][:, ts(rc, RC)],
                                     start=(o == 0), stop=(o == N_IB - 1))
                pt = pts.tile([P, RC], BF16, tag=f"pt{kt}", name=f"pt{kt}")
                nc.scalar.activation(pt, acc_st,
                                     mybir.ActivationFunctionType.Exp,
                                     scale=SCALE)
                base = rc * RC - kt * P
                if base < P - 1:  # tile straddles the causal diagonal
                    nc.gpsimd.affine_select(
                        out=pt, in_=pt,
                        compare_op=mybir.AluOpType.is_ge, fill=0.0,
                        base=base, channel_multiplier=-1, pattern=[[1, RC]])
                nc.tensor.matmul(lp, lhsT=ones, rhs=pt,
                                 start=(n == 0), stop=(n == len(kts) - 1))
                pt_tiles[(kt, rc)] = pt
            return lp

        def emit_pv(rc, lp):
            kts = _kept_kts(rc)
            for db in range(N_IB):
                acc_pv = ps.tile([P, RC], F32, tag="mm", name="acc_pv")
                for n, kt in enumerate(kts):
                    nc.tensor.matmul(acc_pv,
                                     lhsT=v_sb[kt][:, ts(db, P)],
                                     rhs=pt_tiles[(kt, rc)],
                                     start=(n == 0), stop=(n == len(kts) - 1))
                o_sb = outp.tile([P, RC], F32, tag="osb", name="osb")
                nc.vector.tensor_copy(o_sb, acc_pv)
                nc.sync.dma_start(out=ot[ts(db, P), ts(rc, RC)], in_=o_sb)
            l_sb = outp.tile([1, RC], F32, tag="lsb", name="lsb")
            nc.vector.tensor_copy(l_sb, lp)
            nc.sync.dma_start(out=ls[0:1, ts(rc, RC)], in_=l_sb)

        # software-pipelined emission: ST(rc+1) overlaps exp/mask of rc
        lp0 = emit_st(0)
        lp1 = emit_st(1)
        emit_pv(0, lp0)
        lp2 = emit_st(2)
        emit_pv(1, lp1)
        lp3 = emit_st(3)
        emit_pv(2, lp2)
        emit_pv(3, lp3)


_NC_CACHE = {}


def _get_nc():
    if "nc" not in _NC_CACHE:
        nc = bacc.Bacc("TRN2", target_bir_lowering=False, debug=False,
                       enable_asserts=False, num_devices=N_CORES)
        xt = nc.dram_tensor("xt", [D, NQ], BF16, kind="ExternalInput").ap()
        wqt = nc.dram_tensor("wqt", [D, D], BF16, kind="ExternalInput").ap()
        wkt = nc.dram_tensor("wkt", [D, D], BF16, kind="ExternalInput").ap()
        wvt = nc.dram_tensor("wvt", [D, D], BF16, kind="ExternalInput").ap()
        ot = nc.dram_tensor("ot", [D, NQ], F32, kind="ExternalOutput").ap()
        ls = nc.dram_tensor("ls", [1, NQ], F32, kind="ExternalOutput").ap()
        with tile.TileContext(nc) as tc:
            _emit(nc, tc, xt, wqt, wkt, wvt, ot, ls)
        nc.compile()
        _NC_CACHE["nc"] = nc
    return _NC_CACHE["nc"]


def make_in_maps(x, w_query, w_key, w_value):
    bf = ml_dtypes.bfloat16
    wqt = np.ascontiguousarray(np.asarray(w_query).T).astype(bf)
    wkt = np.ascontiguousarray(np.asarray(w_key).T).astype(bf)
    wvt = np.ascontiguousarray(np.asarray(w_value).T).astype(bf)
    in_maps = []
    for c in range(N_CORES):
        b, kh = c // 2, c % 2
        rows = (np.arange(NQ) + kh * H) % S  # slots past S wrap to junk
        xt_np = np.ascontiguousarray(np.asarray(x)[b, rows].T).astype(bf)
        in_maps.append({"xt": xt_np, "wqt": wqt, "wkt": wkt, "wvt": wvt})
    return in_maps


def merge_outputs(results):
    num = np.zeros((B, S, D), np.float32)
    den = np.zeros((B, S), np.float32)
    for c in range(N_CORES):
        b, kh = c // 2, c % 2
        otc = np.asarray(results[c]["ot"])   # [D, NQ]
        lc = np.asarray(results[c]["ls"])[0]  # [NQ]
        nvalid = S - kh * H
        num[b, kh * H:] += otc.T[:nvalid]
        den[b, kh * H:] += lc[:nvalid]
    return (num / den[:, :, None]).astype(np.float32)


def kernel(x, w_query, w_key, w_value, _trace=False):
    nc = _get_nc()
    in_maps = make_in_maps(x, w_query, w_key, w_value)
    res = bass_utils.run_bass_kernel_spmd(
        nc, in_maps, core_ids=list(range(N_CORES)), trace=_trace)
    out = merge_outputs(res.results)
    if _trace:
        kernel.last_result = res
    return out


# revision 2
# speedup vs baseline: 1.0314x; 1.0314x over previous
"""Causal single-head attention (B=4, S=2048, D=1024) on 8 Trainium2 cores.

Sharding: 8 cores = (batch b, key-half kh). Core (b, kh) owns keys/values
for rows [kh*1024, kh*1024+1024) of batch b and computes partial (un-
normalized) attention for ALL queries that can see those keys. Queries are
fed "aligned": query slot r corresponds to global row kh*1024 + r, so the
causal condition is always r >= j (j = local key index) — identical on
every core, letting one SPMD program serve both key halves with a purely
compile-time block-triangular mask. Slots past the end of the sequence
(kh=1, r >= 1024) compute junk that the host discards.

Softmax uses no max-subtraction (logits are O(1) for this problem:
|score/32| < ~4), so per-core partials are just num = exp(S)·V and
l = sum(exp(S)); the host merges halves with num/den addition and one
divide. This is mathematically identical to the reference softmax.

On-chip layout (all matmul operands bf16, fp32 PSUM accumulation):
  xt  = x_core^T              [D=1024, NQ=2048]  (keys are cols 0:1024)
  QT[o,r], KT[o,j] transposed; V[j,d] natural — chosen so every matmul
  is a plain lhsT.T @ rhs with no on-chip transposes:
    KT = wkT.T @ xt,  QT = wqT.T @ xt,  V = xt.T @ wvT
    ST = KT.T @ QT    (scores transposed: partition=key, free=query)
    PT = exp(ST/32) masked to r>=j;  OT = V.T @ PT;  l = 1s.T @ PT
"""

import sys

sys.path.insert(0, "/opt/trn_rl_repo")

from contextlib import ExitStack

import ml_dtypes
import numpy as np

import concourse.bass as bass  # noqa: F401  (engine types resolve via bacc)
import concourse.mybir as mybir
import concourse.tile as tile
from concourse import bacc, bass_utils
from concourse.bass import ts

BF16 = mybir.dt.bfloat16
F32 = mybir.dt.float32

P = 128            # partitions
D = 1024           # model dim (d_in == d_out)
NQ = 2048          # query slots per core
NK = 1024          # keys per core
RC = 512           # query-chunk (matmul moving-dim) size
N_RC = NQ // RC    # 4
N_KT = NK // P     # 8 key tiles
N_IB = D // P      # 8 contraction blocks
SCALE = 1.0 / 32.0 # 1/sqrt(D)

N_CORES = 8
B, S = 4, 2048
H = S // 2


def _kept_kts(rc):
    # (kt, rc) pairs where some query in the chunk can see some key of the
    # tile: min r = rc*RC, max j = kt*P + 127; causal keeps r >= j.
    return [kt for kt in range(N_KT) if kt * P <= rc * RC + RC - 1]


def _emit(nc, tc, xt, wqt, wkt, wvt, ot, ls):
    with ExitStack() as ctx:
        sb = ctx.enter_context(tc.tile_pool(name="sb", bufs=1))
        pts = ctx.enter_context(tc.tile_pool(name="pts", bufs=2))
        outp = ctx.enter_context(tc.tile_pool(name="outp", bufs=3))
        ps = ctx.enter_context(tc.tile_pool(name="ps", bufs=4, space="PSUM"))
        psl = ctx.enter_context(tc.tile_pool(name="psl", bufs=2, space="PSUM"))

        ones = sb.tile([P, 1], BF16, tag="ones", name="ones")
        nc.vector.memset(ones, 1.0)

        # ---- input loads ----
        # Emission order = consumption order, so the first KT matmul can
        # start ~1.5us in (needs only wk[0] + xt[0] low half) instead of
        # stalling on the full 10MB input load.
        xt_sb = [sb.tile([P, NQ], BF16, tag=f"xt{i}", name=f"xt{i}")
                 for i in range(N_IB)]
        w_sb = {nm: [sb.tile([P, D], BF16, tag=f"{nm}{i}", name=f"{nm}{i}")
                     for i in range(N_IB)]
                for nm in ("wk", "wq", "wv")}
        for i in range(N_IB):
            nc.sync.dma_start(out=w_sb["wk"][i], in_=wkt[ts(i, P), :])
            nc.sync.dma_start(out=xt_sb[i][:, 0:NK], in_=xt[ts(i, P), 0:NK])
        for i in range(N_IB):
            nc.sync.dma_start(out=w_sb["wq"][i], in_=wqt[ts(i, P), :])
            nc.sync.dma_start(out=xt_sb[i][:, NK:NQ], in_=xt[ts(i, P), NK:NQ])
        for i in range(N_IB):
            nc.sync.dma_start(out=w_sb["wv"][i], in_=wvt[ts(i, P), :])

        # ---- projections ----
        kt_sb = [sb.tile([P, NK], BF16, tag=f"ktk{o}", name=f"ktk{o}")
                 for o in range(N_IB)]
        for o in range(N_IB):
            for jc in range(NK // RC):
                acc_kt = ps.tile([P, RC], F32, tag="mm", name="acc_kt")
                for i in range(N_IB):
                    nc.tensor.matmul(acc_kt,
                                     lhsT=w_sb["wk"][i][:, ts(o, P)],
                                     rhs=xt_sb[i][:, ts(jc, RC)],
                                     start=(i == 0), stop=(i == N_IB - 1))
                nc.vector.tensor_copy(kt_sb[o][:, ts(jc, RC)], acc_kt)

        qt_sb = [sb.tile([P, NQ], BF16, tag=f"qtq{o}", name=f"qtq{o}")
                 for o in range(N_IB)]
        for rc in range(N_RC):  # rc outer: rc 0/1 only need xt low half
            for o in range(N_IB):
                acc_qt = ps.tile([P, RC], F32, tag="mm", name="acc_qt")
                for i in range(N_IB):
                    nc.tensor.matmul(acc_qt,
                                     lhsT=w_sb["wq"][i][:, ts(o, P)],
                                     rhs=xt_sb[i][:, ts(rc, RC)],
                                     start=(i == 0), stop=(i == N_IB - 1))
                nc.vector.tensor_copy(qt_sb[o][:, ts(rc, RC)], acc_qt)

        v_sb = [sb.tile([P, D], BF16, tag=f"vj{j}", name=f"vj{j}")
                for j in range(N_KT)]
        for j in range(N_KT):
            for dc in range(D // RC):
                acc_v = ps.tile([P, RC], F32, tag="mm", name="acc_v")
                for i in range(N_IB):
                    nc.tensor.matmul(acc_v,
                                     lhsT=xt_sb[i][:, ts(j, P)],
                                     rhs=w_sb["wv"][i][:, ts(dc, RC)],
                                     start=(i == 0), stop=(i == N_IB - 1))
                nc.vector.tensor_copy(v_sb[j][:, ts(dc, RC)], acc_v)

        # ---- attention ----
        pt_tiles = {}

        def emit_st(rc):
            kts = _kept_kts(rc)
            lp = psl.tile([1, RC], F32, tag="lp", name="lp")
            for n, kt in enumerate(kts):
                acc_st = ps.tile([P, RC], F32, tag="mm", name="acc_st")
                for o in range(N_IB):
                    nc.tensor.matmul(acc_st,
                                     lhsT=kt_sb[o][:, ts(kt, P)],
                                     rhs=qt_sb[o][:, ts(rc, RC)],
                                     start=(o == 0), stop=(o == N_IB - 1))
                pt = pts.tile([P, RC], BF16, tag=f"pt{kt}", name=f"pt{kt}")
                nc.scalar.activation(pt, acc_st,
                                     mybir.ActivationFunctionType.Exp,
                                     scale=SCALE)
                base = rc * RC - kt * P
                if base < P - 1:  # tile straddles the causal diagonal
                    nc.gpsimd.affine_select(
                        out=pt, in_=pt,
                        compare_op=mybir.AluOpType.is_ge, fill=0.0,
                        base=base, channel_multiplier=-1, pattern=[[1, RC]])
                nc.tensor.matmul(lp, lhsT=ones, rhs=pt,
                                 start=(n == 0), stop=(n == len(kts) - 1))
                pt_tiles[(kt, rc)] = pt
            return lp

        def emit_pv(rc, lp):
            kts = _kept_kts(rc)
            for db in range(N_IB):
                acc_pv = ps.tile([P, RC], F32, tag="mm", name="acc_pv")
                for n, kt in enumerate(kts):
                    nc.tensor.matmul(acc_pv,
                                     lhsT=v_sb[kt][:, ts(db, P)],
                                     rhs=pt_tiles[(kt, rc)],
                                     start=(n == 0), stop=(n == len(kts) - 1))
                o_sb = outp.tile([P, RC], F32, tag="osb", name="osb")
                nc.vector.tensor_copy(o_sb, acc_pv)
                nc.sync.dma_start(out=ot[ts(db, P), ts(rc, RC)], in_=o_sb)
            l_sb = outp.tile([1, RC], F32, tag="lsb", name="lsb")
            nc.vector.tensor_copy(l_sb, lp)
            nc.sync.dma_start(out=ls[0:1, ts(rc, RC)], in_=l_sb)

        # software-pipelined emission: ST(rc+1) overlaps exp/mask of rc
        lp0 = emit_st(0)
        lp1 = emit_st(1)
        emit_pv(0, lp0)
        lp2 = emit_st(2)
        emit_pv(1, lp1)
        lp3 = emit_st(3)
        emit_pv(2, lp2)
        emit_pv(3, lp3)


_NC_CACHE = {}


def _get_nc():
    if "nc" not in _NC_CACHE:
        nc = bacc.Bacc("TRN2", target_bir_lowering=False, debug=False,
                       enable_asserts=False, num_devices=N_CORES)
        xt = nc.dram_tensor("xt", [D, NQ], BF16, kind="ExternalInput").ap()
        wqt = nc.dram_tensor("wqt", [D, D], BF16, kind="ExternalInput").ap()
        wkt = nc.dram_tensor("wkt", [D, D], BF16, kind="ExternalInput").ap()
        wvt = nc.dram_tensor("wvt", [D, D], BF16, kind="ExternalInput").ap()
        ot = nc.dram_tensor("ot", [D, NQ], F32, kind="ExternalOutput").ap()
        ls = nc.dram_tensor("ls", [1, NQ], F32, kind="ExternalOutput").ap()
        with tile.TileContext(nc) as tc:
            _emit(nc, tc, xt, wqt, wkt, wvt, ot, ls)
        nc.compile()
        _NC_CACHE["nc"] = nc
    return _NC_CACHE["nc"]


def make_in_maps(x, w_query, w_key, w_value):
    bf = ml_dtypes.bfloat16
    wqt = np.ascontiguousarray(np.asarray(w_query).T).astype(bf)
    wkt = np.ascontiguousarray(np.asarray(w_key).T).astype(bf)
    wvt = np.ascontiguousarray(np.asarray(w_value).T).astype(bf)
    in_maps = []
    for c in range(N_CORES):
        b, kh = c // 2, c % 2
        rows = (np.arange(NQ) + kh * H) % S  # slots past S wrap to junk
        xt_np = np.ascontiguousarray(np.asarray(x)[b, rows].T).astype(bf)
        in_maps.append({"xt": xt_np, "wqt": wqt, "wkt": wkt, "wvt": wvt})
    return in_maps


def merge_outputs(results):
    num = np.zeros((B, S, D), np.float32)
    den = np.zeros((B, S), np.float32)
    for c in range(N_CORES):
        b, kh = c // 2, c % 2
        otc = np.asarray(results[c]["ot"])   # [D, NQ]
        lc = np.asarray(results[c]["ls"])[0]  # [NQ]
        nvalid = S - kh * H
        num[b, kh * H:] += otc.T[:nvalid]
        den[b, kh * H:] += lc[:nvalid]
    return (num / den[:, :, None]).astype(np.float32)


def kernel(x, w_query, w_key, w_value, _trace=False):
    nc = _get_nc()
    in_maps = make_in_maps(x, w_query, w_key, w_value)
    res = bass_utils.run_bass_kernel_spmd(
        nc, in_maps, core_ids=list(range(N_CORES)), trace=_trace)
    out = merge_outputs(res.results)
    if _trace:
        kernel.last_result = res
    return out


# revision 3
# speedup vs baseline: 1.1572x; 1.1220x over previous
"""Causal single-head attention (B=4, S=2048, D=1024) on 8 Trainium2 cores.

Sharding: 8 cores = (batch b, stripe-set eta). Core (b, eta) owns four
interleaved key stripes of 256 rows at global offsets 512k + 256*eta
(k = 0..3) of batch b, stored locally stripe-major (local key
ell in [256k, 256k+256) -> global 512k + 256*eta + ell%256). Queries are
fed "aligned" with base beta = 256*eta: query col c corresponds to global
row beta + c. Then the causal condition for local key tile kt vs query
chunk rc is c >= 512*(kt//2) + 128*(kt%2) + x — identical on every core,
so one SPMD program serves both stripe sets with a purely compile-time
block mask, and score blocks with kt >= 2*(rc+1) are skipped outright
(20 of 32 blocks kept vs 28 for a half-split). Cols past the sequence end
(eta=1, c >= 1792) compute junk that the host discards.

Softmax uses no max-subtraction (logits are O(1) for this problem:
|score/32| < ~4), so per-core partials are just num = exp(S)·V and
l = sum(exp(S)); the host merges halves with num/den addition and one
divide. This is mathematically identical to the reference softmax.

On-chip layout (all matmul operands bf16, fp32 PSUM accumulation):
  xt  = x_core^T              [D=1024, NQ=2048]  (keys are cols 0:1024)
  QT[o,r], KT[o,j] transposed; V[j,d] natural — chosen so every matmul
  is a plain lhsT.T @ rhs with no on-chip transposes:
    KT = wkT.T @ xt,  QT = wqT.T @ xt,  V = xt.T @ wvT
    ST = KT.T @ QT    (scores transposed: partition=key, free=query)
    PT = exp(ST/32) masked to r>=j;  OT = V.T @ PT;  l = 1s.T @ PT
"""

import sys

sys.path.insert(0, "/opt/trn_rl_repo")

from contextlib import ExitStack

import ml_dtypes
import numpy as np

import concourse.bass as bass  # noqa: F401  (engine types resolve via bacc)
import concourse.mybir as mybir
import concourse.tile as tile
from concourse import bacc, bass_utils
from concourse.bass import ts

BF16 = mybir.dt.bfloat16
F32 = mybir.dt.float32

P = 128            # partitions
D = 1024           # model dim (d_in == d_out)
NQ = 2048          # query slots per core
NK = 1024          # keys per core
RC = 512           # query-chunk (matmul moving-dim) size
N_RC = NQ // RC    # 4
N_KT = NK // P     # 8 key tiles
N_IB = D // P      # 8 contraction blocks
SCALE = 1.0 / 32.0 # 1/sqrt(D)

N_CORES = 8
B, S = 4, 2048
STRIPE = 256


def _kept_kts(rc):
    # key tile kt (stripe k = kt//2) is visible to query chunk rc iff some
    # col c in [rc*512, rc*512+512) has c >= 512*(kt//2) + 128*(kt%2).
    return [kt for kt in range(N_KT) if kt < 2 * (rc + 1)]


def _mask_base(rc, kt):
    # keep when  y + 512*rc >= x + 512*(kt//2) + 128*(kt%2)
    return RC * rc - RC * (kt // 2) - P * (kt % 2)


def _emit(nc, tc, xt, wqt, wkt, wvt, ot, ls):
    with ExitStack() as ctx:
        sb = ctx.enter_context(tc.tile_pool(name="sb", bufs=1))
        pts = ctx.enter_context(tc.tile_pool(name="pts", bufs=2))
        outp = ctx.enter_context(tc.tile_pool(name="outp", bufs=3))
        ps = ctx.enter_context(tc.tile_pool(name="ps", bufs=4, space="PSUM"))
        psl = ctx.enter_context(tc.tile_pool(name="psl", bufs=2, space="PSUM"))

        ones = sb.tile([P, 1], BF16, tag="ones", name="ones")
        nc.vector.memset(ones, 1.0)

        # ---- input loads ----
        # Emission order = consumption order, so the first KT matmul can
        # start ~1.5us in (needs only wk[0] + xt[0] low half) instead of
        # stalling on the full 10MB input load.
        xt_sb = [sb.tile([P, NQ], BF16, tag=f"xt{i}", name=f"xt{i}")
                 for i in range(N_IB)]
        w_sb = {nm: [sb.tile([P, D], BF16, tag=f"{nm}{i}", name=f"{nm}{i}")
                     for i in range(N_IB)]
                for nm in ("wk", "wq", "wv")}
        xkv_sb = [sb.tile([P, NK], BF16, tag=f"xkv{i}", name=f"xkv{i}")
                  for i in range(N_IB)]
        for i in range(N_IB):
            nc.sync.dma_start(out=w_sb["wk"][i], in_=wkt[ts(i, P), :])
            # gather the 4 key stripes (cols 512k..512k+256 of xt) into a
            # dense [128, 1024] stripe-major kv tile
            nc.sync.dma_start(
                out=xkv_sb[i].rearrange("p (k c) -> p k c", c=256),
                in_=xt[ts(i, P), :].rearrange("p (k c) -> p k c", c=512)[:, :, 0:256])
        for i in range(N_IB):
            nc.sync.dma_start(out=xt_sb[i][:, 0:NK], in_=xt[ts(i, P), 0:NK])
            nc.sync.dma_start(out=w_sb["wq"][i], in_=wqt[ts(i, P), :])
        for i in range(N_IB):
            nc.sync.dma_start(out=xt_sb[i][:, NK:NQ], in_=xt[ts(i, P), NK:NQ])
            nc.sync.dma_start(out=w_sb["wv"][i], in_=wvt[ts(i, P), :])

        # ---- projections ----
        kt_sb = [sb.tile([P, NK], BF16, tag=f"ktk{o}", name=f"ktk{o}")
                 for o in range(N_IB)]
        for o in range(N_IB):
            for jc in range(NK // RC):
                acc_kt = ps.tile([P, RC], F32, tag="mm", name="acc_kt")
                for i in range(N_IB):
                    nc.tensor.matmul(acc_kt,
                                     lhsT=w_sb["wk"][i][:, ts(o, P)],
                                     rhs=xkv_sb[i][:, ts(jc, RC)],
                                     start=(i == 0), stop=(i == N_IB - 1))
                nc.vector.tensor_copy(kt_sb[o][:, ts(jc, RC)], acc_kt)

        qt_sb = [sb.tile([P, NQ], BF16, tag=f"qtq{o}", name=f"qtq{o}")
                 for o in range(N_IB)]
        for rc in range(N_RC):  # rc outer: rc 0/1 only need xt low half
            for o in range(N_IB):
                acc_qt = ps.tile([P, RC], F32, tag="mm", name="acc_qt")
                for i in range(N_IB):
                    nc.tensor.matmul(acc_qt,
                                     lhsT=w_sb["wq"][i][:, ts(o, P)],
                                     rhs=xt_sb[i][:, ts(rc, RC)],
                                     start=(i == 0), stop=(i == N_IB - 1))
                nc.vector.tensor_copy(qt_sb[o][:, ts(rc, RC)], acc_qt)

        v_sb = [sb.tile([P, D], BF16, tag=f"vj{j}", name=f"vj{j}")
                for j in range(N_KT)]
        for j in range(N_KT):
            for dc in range(D // RC):
                acc_v = ps.tile([P, RC], F32, tag="mm", name="acc_v")
                for i in range(N_IB):
                    nc.tensor.matmul(acc_v,
                                     lhsT=xkv_sb[i][:, ts(j, P)],
                                     rhs=w_sb["wv"][i][:, ts(dc, RC)],
                                     start=(i == 0), stop=(i == N_IB - 1))
                nc.vector.tensor_copy(v_sb[j][:, ts(dc, RC)], acc_v)

        # ---- attention ----
        pt_tiles = {}

        def emit_st(rc):
            kts = _kept_kts(rc)
            lp = psl.tile([1, RC], F32, tag="lp", name="lp")
            for n, kt in enumerate(kts):
                acc_st = ps.tile([P, RC], F32, tag="mm", name="acc_st")
                for o in range(N_IB):
                    nc.tensor.matmul(acc_st,
                                     lhsT=kt_sb[o][:, ts(kt, P)],
                                     rhs=qt_sb[o][:, ts(rc, RC)],
                                     start=(o == 0), stop=(o == N_IB - 1))
                pt = pts.tile([P, RC], BF16, tag=f"pt{kt}", name=f"pt{kt}")
                nc.scalar.activation(pt, acc_st,
                                     mybir.ActivationFunctionType.Exp,
                                     scale=SCALE)
                base = _mask_base(rc, kt)
                if base < P - 1:  # tile straddles the causal diagonal
                    nc.gpsimd.affine_select(
                        out=pt, in_=pt,
                        compare_op=mybir.AluOpType.is_ge, fill=0.0,
                        base=base, channel_multiplier=-1, pattern=[[1, RC]])
                nc.tensor.matmul(lp, lhsT=ones, rhs=pt,
                                 start=(n == 0), stop=(n == len(kts) - 1))
                pt_tiles[(kt, rc)] = pt
            return lp

        def emit_pv(rc, lp):
            kts = _kept_kts(rc)
            for db in range(N_IB):
                acc_pv = ps.tile([P, RC], F32, tag="mm", name="acc_pv")
                for n, kt in enumerate(kts):
                    nc.tensor.matmul(acc_pv,
                                     lhsT=v_sb[kt][:, ts(db, P)],
                                     rhs=pt_tiles[(kt, rc)],
                                     start=(n == 0), stop=(n == len(kts) - 1))
                o_sb = outp.tile([P, RC], F32, tag="osb", name="osb")
                nc.vector.tensor_copy(o_sb, acc_pv)
                nc.sync.dma_start(out=ot[ts(db, P), ts(rc, RC)], in_=o_sb)
            l_sb = outp.tile([1, RC], F32, tag="lsb", name="lsb")
            nc.vector.tensor_copy(l_sb, lp)
            nc.sync.dma_start(out=ls[0:1, ts(rc, RC)], in_=l_sb)

        # software-pipelined emission: ST(rc+1) overlaps exp/mask of rc
        lp0 = emit_st(0)
        lp1 = emit_st(1)
        emit_pv(0, lp0)
        lp2 = emit_st(2)
        emit_pv(1, lp1)
        lp3 = emit_st(3)
        emit_pv(2, lp2)
        emit_pv(3, lp3)


_NC_CACHE = {}


def _get_nc():
    if "nc" not in _NC_CACHE:
        nc = bacc.Bacc("TRN2", target_bir_lowering=False, debug=False,
                       enable_asserts=False, num_devices=N_CORES)
        xt = nc.dram_tensor("xt", [D, NQ], BF16, kind="ExternalInput").ap()
        wqt = nc.dram_tensor("wqt", [D, D], BF16, kind="ExternalInput").ap()
        wkt = nc.dram_tensor("wkt", [D, D], BF16, kind="ExternalInput").ap()
        wvt = nc.dram_tensor("wvt", [D, D], BF16, kind="ExternalInput").ap()
        ot = nc.dram_tensor("ot", [D, NQ], F32, kind="ExternalOutput").ap()
        ls = nc.dram_tensor("ls", [1, NQ], F32, kind="ExternalOutput").ap()
        with tile.TileContext(nc) as tc:
            _emit(nc, tc, xt, wqt, wkt, wvt, ot, ls)
        nc.compile()
        _NC_CACHE["nc"] = nc
    return _NC_CACHE["nc"]


def make_in_maps(x, w_query, w_key, w_value):
    bf = ml_dtypes.bfloat16
    wqt = np.ascontiguousarray(np.asarray(w_query).T).astype(bf)
    wkt = np.ascontiguousarray(np.asarray(w_key).T).astype(bf)
    wvt = np.ascontiguousarray(np.asarray(w_value).T).astype(bf)
    in_maps = []
    for c in range(N_CORES):
        b, eta = c // 2, c % 2
        rows = (np.arange(NQ) + eta * STRIPE) % S  # cols past S wrap to junk
        xt_np = np.ascontiguousarray(np.asarray(x)[b, rows].T).astype(bf)
        in_maps.append({"xt": xt_np, "wqt": wqt, "wkt": wkt, "wvt": wvt})
    return in_maps


def merge_outputs(results):
    num = np.zeros((B, S, D), np.float32)
    den = np.zeros((B, S), np.float32)
    for c in range(N_CORES):
        b, eta = c // 2, c % 2
        otc = np.asarray(results[c]["ot"])   # [D, NQ]
        lc = np.asarray(results[c]["ls"])[0]  # [NQ]
        beta = eta * STRIPE
        nvalid = S - beta
        num[b, beta:] += otc.T[:nvalid]
        den[b, beta:] += lc[:nvalid]
    return (num / den[:, :, None]).astype(np.float32)


def kernel(x, w_query, w_key, w_value, _trace=False):
    nc = _get_nc()
    in_maps = make_in_maps(x, w_query, w_key, w_value)
    res = bass_utils.run_bass_kernel_spmd(
        nc, in_maps, core_ids=list(range(N_CORES)), trace=_trace)
    out = merge_outputs(res.results)
    if _trace:
        kernel.last_result = res
    return out


# revision 4
# speedup vs baseline: 1.2041x; 1.0406x over previous
"""Causal single-head attention (B=4, S=2048, D=1024) on 8 Trainium2 cores.

Sharding: 8 cores = (batch b, stripe-set eta). Core (b, eta) owns four
interleaved key stripes of 256 rows at global offsets 512k + 256*eta
(k = 0..3) of batch b, stored locally stripe-major (local key
ell in [256k, 256k+256) -> global 512k + 256*eta + ell%256). Queries are
fed "aligned" with base beta = 256*eta: query col c corresponds to global
row beta + c. Then the causal condition for local key tile kt vs query
chunk rc is c >= 512*(kt//2) + 128*(kt%2) + x — identical on every core,
so one SPMD program serves both stripe sets with a purely compile-time
block mask, and score blocks with kt >= 2*(rc+1) are skipped outright
(20 of 32 blocks kept vs 28 for a half-split). Cols past the sequence end
(eta=1, c >= 1792) compute junk that the host discards.

Softmax uses no max-subtraction (logits are O(1) for this problem:
|score/32| < ~4), so per-core partials are just num = exp(S)·V and
l = sum(exp(S)); the host merges halves with num/den addition and one
divide. This is mathematically identical to the reference softmax.

On-chip layout (all matmul operands bf16, fp32 PSUM accumulation):
  xt  = x_core^T              [D=1024, NQ=2048]  (keys are cols 0:1024)
  QT[o,r], KT[o,j] transposed; V[j,d] natural — chosen so every matmul
  is a plain lhsT.T @ rhs with no on-chip transposes:
    KT = wkT.T @ xt,  QT = wqT.T @ xt,  V = xt.T @ wvT
    ST = KT.T @ QT    (scores transposed: partition=key, free=query)
    PT = exp(ST/32) masked to r>=j;  OT = V.T @ PT;  l = 1s.T @ PT
"""

import sys

sys.path.insert(0, "/opt/trn_rl_repo")

from contextlib import ExitStack

import ml_dtypes
import numpy as np

import concourse.bass as bass  # noqa: F401  (engine types resolve via bacc)
import concourse.mybir as mybir
import concourse.tile as tile
from concourse import bacc, bass_utils
from concourse.bass import ts

BF16 = mybir.dt.bfloat16
F32 = mybir.dt.float32

P = 128            # partitions
D = 1024           # model dim (d_in == d_out)
NQ = 2048          # query slots per core
NK = 1024          # keys per core
RC = 512           # query-chunk (matmul moving-dim) size
N_RC = NQ // RC    # 4
N_KT = NK // P     # 8 key tiles
N_IB = D // P      # 8 contraction blocks
SCALE = 1.0 / 32.0 # 1/sqrt(D)

N_CORES = 8
B, S = 4, 2048
STRIPE = 256


def _kept_kts(rc):
    # key tile kt (stripe k = kt//2) is visible to query chunk rc iff some
    # col c in [rc*512, rc*512+512) has c >= 512*(kt//2) + 128*(kt%2).
    return [kt for kt in range(N_KT) if kt < 2 * (rc + 1)]


def _mask_base(rc, kt):
    # keep when  y + 512*rc >= x + 512*(kt//2) + 128*(kt%2)
    return RC * rc - RC * (kt // 2) - P * (kt % 2)


def _emit(nc, tc, xt, wqt, wkt, wvt, ot, ls):
    with ExitStack() as ctx:
        sb = ctx.enter_context(tc.tile_pool(name="sb", bufs=1))
        pts = ctx.enter_context(tc.tile_pool(name="pts", bufs=2))
        outp = ctx.enter_context(tc.tile_pool(name="outp", bufs=3))
        ps = ctx.enter_context(tc.tile_pool(name="ps", bufs=6, space="PSUM"))
        psl = ctx.enter_context(tc.tile_pool(name="psl", bufs=2, space="PSUM"))

        ones = sb.tile([P, 1], BF16, tag="ones", name="ones")
        nc.vector.memset(ones, 1.0)

        # ---- input loads ----
        # Emission order = consumption order, so the first KT matmul can
        # start ~1.5us in (needs only wk[0] + xt[0] low half) instead of
        # stalling on the full 10MB input load.
        xt_sb = [sb.tile([P, NQ], BF16, tag=f"xt{i}", name=f"xt{i}")
                 for i in range(N_IB)]
        w_sb = {nm: [sb.tile([P, D], BF16, tag=f"{nm}{i}", name=f"{nm}{i}")
                     for i in range(N_IB)]
                for nm in ("wk", "wq", "wv")}
        xkv_sb = [sb.tile([P, NK], BF16, tag=f"xkv{i}", name=f"xkv{i}")
                  for i in range(N_IB)]
        for i in range(N_IB):
            nc.sync.dma_start(out=w_sb["wk"][i], in_=wkt[ts(i, P), :])
            # gather the 4 key stripes (cols 512k..512k+256 of xt) into a
            # dense [128, 1024] stripe-major kv tile
            nc.sync.dma_start(
                out=xkv_sb[i].rearrange("p (k c) -> p k c", c=256),
                in_=xt[ts(i, P), :].rearrange("p (k c) -> p k c", c=512)[:, :, 0:256])
        for i in range(N_IB):
            nc.sync.dma_start(out=w_sb["wv"][i], in_=wvt[ts(i, P), :])
        for i in range(N_IB):
            nc.sync.dma_start(out=w_sb["wq"][i], in_=wqt[ts(i, P), :])
        for i in range(N_IB):
            nc.sync.dma_start(out=xt_sb[i][:, 0:NK], in_=xt[ts(i, P), 0:NK])
        for i in range(N_IB):
            nc.sync.dma_start(out=xt_sb[i][:, NK:NQ], in_=xt[ts(i, P), NK:NQ])

        # ---- projections ----
        # i-major emission in batches of 4 PSUM groups: each arriving DMA
        # block immediately feeds 4 matmuls, and consecutive matmuls with
        # the same stationary operand sit adjacent in the PE stream.
        def proj_phase(groups, dst, lhs_of, rhs_of):
            for gb in range(0, len(groups), 4):
                batch = groups[gb:gb + 4]
                accs = [ps.tile([P, RC], F32, tag="mm", name="acc_p")
                        for _ in batch]
                for i in range(N_IB):
                    for a, g in zip(accs, batch):
                        nc.tensor.matmul(a, lhsT=lhs_of(i, g),
                                         rhs=rhs_of(i, g),
                                         start=(i == 0), stop=(i == N_IB - 1))
                for a, g in zip(accs, batch):
                    nc.vector.tensor_copy(dst(g), a)

        kt_sb = [sb.tile([P, NK], BF16, tag=f"ktk{o}", name=f"ktk{o}")
                 for o in range(N_IB)]
        proj_phase([(o, jc) for o in range(N_IB) for jc in range(NK // RC)],
                   dst=lambda g: kt_sb[g[0]][:, ts(g[1], RC)],
                   lhs_of=lambda i, g: w_sb["wk"][i][:, ts(g[0], P)],
                   rhs_of=lambda i, g: xkv_sb[i][:, ts(g[1], RC)])

        v_sb = [sb.tile([P, D], BF16, tag=f"vj{j}", name=f"vj{j}")
                for j in range(N_KT)]
        proj_phase([(j, dc) for j in range(N_KT) for dc in range(D // RC)],
                   dst=lambda g: v_sb[g[0]][:, ts(g[1], RC)],
                   lhs_of=lambda i, g: xkv_sb[i][:, ts(g[0], P)],
                   rhs_of=lambda i, g: w_sb["wv"][i][:, ts(g[1], RC)])

        qt_sb = [sb.tile([P, NQ], BF16, tag=f"qtq{o}", name=f"qtq{o}")
                 for o in range(N_IB)]
        proj_phase([(o, rc) for o in range(N_IB) for rc in range(N_RC)],
                   dst=lambda g: qt_sb[g[0]][:, ts(g[1], RC)],
                   lhs_of=lambda i, g: w_sb["wq"][i][:, ts(g[0], P)],
                   rhs_of=lambda i, g: xt_sb[i][:, ts(g[1], RC)])

        # ---- attention ----
        pt_tiles = {}

        def emit_st(rc):
            kts = _kept_kts(rc)
            lp = psl.tile([1, RC], F32, tag="lp", name="lp")
            for n, kt in enumerate(kts):
                acc_st = ps.tile([P, RC], F32, tag="mm", name="acc_st")
                for o in range(N_IB):
                    nc.tensor.matmul(acc_st,
                                     lhsT=kt_sb[o][:, ts(kt, P)],
                                     rhs=qt_sb[o][:, ts(rc, RC)],
                                     start=(o == 0), stop=(o == N_IB - 1))
                pt = pts.tile([P, RC], BF16, tag=f"pt{kt}", name=f"pt{kt}")
                nc.scalar.activation(pt, acc_st,
                                     mybir.ActivationFunctionType.Exp,
                                     scale=SCALE)
                base = _mask_base(rc, kt)
                if base < P - 1:  # tile straddles the causal diagonal
                    nc.gpsimd.affine_select(
                        out=pt, in_=pt,
                        compare_op=mybir.AluOpType.is_ge, fill=0.0,
                        base=base, channel_multiplier=-1, pattern=[[1, RC]])
                nc.tensor.matmul(lp, lhsT=ones, rhs=pt,
                                 start=(n == 0), stop=(n == len(kts) - 1))
                pt_tiles[(kt, rc)] = pt
            return lp

        def emit_pv(rc, lp):
            kts = _kept_kts(rc)
            for db in range(N_IB):
                acc_pv = ps.tile([P, RC], F32, tag="mm", name="acc_pv")
                for n, kt in enumerate(kts):
                    nc.tensor.matmul(acc_pv,
                                     lhsT=v_sb[kt][:, ts(db, P)],
                                     rhs=pt_tiles[(kt, rc)],
                                     start=(n == 0), stop=(n == len(kts) - 1))
                o_sb = outp.tile([P, RC], F32, tag="osb", name="osb")
                nc.vector.tensor_copy(o_sb, acc_pv)
                nc.sync.dma_start(out=ot[ts(db, P), ts(rc, RC)], in_=o_sb)
            l_sb = outp.tile([1, RC], F32, tag="lsb", name="lsb")
            nc.vector.tensor_copy(l_sb, lp)
            nc.sync.dma_start(out=ls[0:1, ts(rc, RC)], in_=l_sb)

        # software-pipelined emission: ST(rc+1) overlaps exp/mask of rc
        lp0 = emit_st(0)
        lp1 = emit_st(1)
        emit_pv(0, lp0)
        lp2 = emit_st(2)
        emit_pv(1, lp1)
        lp3 = emit_st(3)
        emit_pv(2, lp2)
        emit_pv(3, lp3)


_NC_CACHE = {}


def _get_nc():
    if "nc" not in _NC_CACHE:
        nc = bacc.Bacc("TRN2", target_bir_lowering=False, debug=False,
                       enable_asserts=False, num_devices=N_CORES)
        xt = nc.dram_tensor("xt", [D, NQ], BF16, kind="ExternalInput").ap()
        wqt = nc.dram_tensor("wqt", [D, D], BF16, kind="ExternalInput").ap()
        wkt = nc.dram_tensor("wkt", [D, D], BF16, kind="ExternalInput").ap()
        wvt = nc.dram_tensor("wvt", [D, D], BF16, kind="ExternalInput").ap()
        ot = nc.dram_tensor("ot", [D, NQ], F32, kind="ExternalOutput").ap()
        ls = nc.dram_tensor("ls", [1, NQ], F32, kind="ExternalOutput").ap()
        with tile.TileContext(nc) as tc:
            _emit(nc, tc, xt, wqt, wkt, wvt, ot, ls)
        nc.compile()
        _NC_CACHE["nc"] = nc
    return _NC_CACHE["nc"]


def make_in_maps(x, w_query, w_key, w_value):
    bf = ml_dtypes.bfloat16
    wqt = np.ascontiguousarray(np.asarray(w_query).T).astype(bf)
    wkt = np.ascontiguousarray(np.asarray(w_key).T).astype(bf)
    wvt = np.ascontiguousarray(np.asarray(w_value).T).astype(bf)
    in_maps = []
    for c in range(N_CORES):
        b, eta = c // 2, c % 2
        rows = (np.arange(NQ) + eta * STRIPE) % S  # cols past S wrap to junk
        xt_np = np.ascontiguousarray(np.asarray(x)[b, rows].T).astype(bf)
        in_maps.append({"xt": xt_np, "wqt": wqt, "wkt": wkt, "wvt": wvt})
    return in_maps


def merge_outputs(results):
    num = np.zeros((B, S, D), np.float32)
    den = np.zeros((B, S), np.float32)
    for c in range(N_CORES):
        b, eta = c // 2, c % 2
        otc = np.asarray(results[c]["ot"])   # [D, NQ]
        lc = np.asarray(results[c]["ls"])[0]  # [NQ]
        beta = eta * STRIPE
        nvalid = S - beta
        num[b, beta:] += otc.T[:nvalid]
        den[b, beta:] += lc[:nvalid]
    return (num / den[:, :, None]).astype(np.float32)


def kernel(x, w_query, w_key, w_value, _trace=False):
    nc = _get_nc()
    in_maps = make_in_maps(x, w_query, w_key, w_value)
    res = bass_utils.run_bass_kernel_spmd(
        nc, in_maps, core_ids=list(range(N_CORES)), trace=_trace)
    out = merge_outputs(res.results)
    if _trace:
        kernel.last_result = res
    return out


# revision 5
# speedup vs baseline: 1.2977x; 1.0777x over previous
"""Causal single-head attention (B=4, S=2048, D=1024) on 8 Trainium2 cores.

Sharding: 8 cores = (batch b, stripe-set eta). Core (b, eta) owns four
interleaved key stripes of 256 rows at global offsets 512k + 256*eta
(k = 0..3) of batch b, stored locally stripe-major (local key
ell in [256k, 256k+256) -> global 512k + 256*eta + ell%256). Queries are
fed "aligned" with base beta = 256*eta: query col c corresponds to global
row beta + c. Then the causal condition for local key tile kt vs query
chunk rc is c >= 512*(kt//2) + 128*(kt%2) + x — identical on every core,
so one SPMD program serves both stripe sets with a purely compile-time
block mask, and score blocks with kt >= 2*(rc+1) are skipped outright
(20 of 32 blocks kept vs 28 for a half-split). Cols past the sequence end
(eta=1, c >= 1792) compute junk that the host discards.

Softmax uses no max-subtraction (logits are O(1) for this problem:
|score/32| < ~4), so per-core partials are just num = exp(S)·V and
l = sum(exp(S)); the host merges halves with num/den addition and one
divide. This is mathematically identical to the reference softmax.

On-chip layout (all matmul operands bf16, fp32 PSUM accumulation):
  xt  = x_core^T              [D=1024, NQ=2048]  (keys are cols 0:1024)
  QT[o,r], KT[o,j] transposed; V[j,d] natural — chosen so every matmul
  is a plain lhsT.T @ rhs with no on-chip transposes:
    KT = wkT.T @ xt,  QT = wqT.T @ xt,  V = xt.T @ wvT
    ST = KT.T @ QT    (scores transposed: partition=key, free=query)
    PT = exp(ST/32) masked to r>=j;  OT = V.T @ PT;  l = 1s.T @ PT
"""

import sys

sys.path.insert(0, "/opt/trn_rl_repo")

from contextlib import ExitStack

import ml_dtypes
import numpy as np

import concourse.bass as bass  # noqa: F401  (engine types resolve via bacc)
import concourse.mybir as mybir
import concourse.tile as tile
from concourse import bacc, bass_utils
from concourse.bass import ts

BF16 = mybir.dt.bfloat16
FP8 = mybir.dt.float8e4
F32 = mybir.dt.float32

P = 128            # partitions
D = 1024           # model dim (d_in == d_out)
NQ = 2048          # query slots per core
NK = 1024          # keys per core
RC = 512           # query-chunk (matmul moving-dim) size
N_RC = NQ // RC    # 4
N_KT = NK // P     # 8 key tiles
N_IB = D // P      # 8 contraction blocks
SCALE = 1.0 / 32.0 # 1/sqrt(D)

N_CORES = 8
B, S = 4, 2048
STRIPE = 256


def _kept_kts(rc):
    # key tile kt (stripe k = kt//2) is visible to query chunk rc iff some
    # col c in [rc*512, rc*512+512) has c >= 512*(kt//2) + 128*(kt%2).
    return [kt for kt in range(N_KT) if kt < 2 * (rc + 1)]


def _mask_base(rc, kt):
    # keep when  y + 512*rc >= x + 512*(kt//2) + 128*(kt%2)
    return RC * rc - RC * (kt // 2) - P * (kt % 2)


def _emit(nc, tc, xt, wqt, wkt, wvt, ot, ls):
    with ExitStack() as ctx:
        sb = ctx.enter_context(tc.tile_pool(name="sb", bufs=1))
        pts = ctx.enter_context(tc.tile_pool(name="pts", bufs=2))
        outp = ctx.enter_context(tc.tile_pool(name="outp", bufs=3))
        ps = ctx.enter_context(tc.tile_pool(name="ps", bufs=6, space="PSUM"))
        psl = ctx.enter_context(tc.tile_pool(name="psl", bufs=2, space="PSUM"))

        ones = sb.tile([P, 1], BF16, tag="ones", name="ones")
        nc.vector.memset(ones, 1.0)

        # ---- input loads ----
        # Emission order = consumption order, so the first KT matmul can
        # start ~1.5us in (needs only wk[0] + xt[0] low half) instead of
        # stalling on the full 10MB input load.
        xt_sb = [sb.tile([P, NQ], BF16, tag=f"xt{i}", name=f"xt{i}")
                 for i in range(N_IB)]
        w_sb = {nm: [sb.tile([P, D], BF16, tag=f"{nm}{i}", name=f"{nm}{i}")
                     for i in range(N_IB)]
                for nm in ("wk", "wq", "wv")}
        xkv_sb = [sb.tile([P, NK], BF16, tag=f"xkv{i}", name=f"xkv{i}")
                  for i in range(N_IB)]
        for i in range(N_IB):
            nc.sync.dma_start(out=w_sb["wk"][i], in_=wkt[ts(i, P), :])
            # gather the 4 key stripes (cols 512k..512k+256 of xt) into a
            # dense [128, 1024] stripe-major kv tile
            nc.sync.dma_start(
                out=xkv_sb[i].rearrange("p (k c) -> p k c", c=256),
                in_=xt[ts(i, P), :].rearrange("p (k c) -> p k c", c=512)[:, :, 0:256])
        for i in range(N_IB):
            nc.sync.dma_start(out=w_sb["wv"][i], in_=wvt[ts(i, P), :])
        for i in range(N_IB):
            nc.sync.dma_start(out=w_sb["wq"][i], in_=wqt[ts(i, P), :])
        for i in range(N_IB):
            nc.sync.dma_start(out=xt_sb[i][:, 0:NK], in_=xt[ts(i, P), 0:NK])
        for i in range(N_IB):
            nc.sync.dma_start(out=xt_sb[i][:, NK:NQ], in_=xt[ts(i, P), NK:NQ])

        # ---- projections ----
        # i-major emission in batches of 4 PSUM groups: each arriving DMA
        # block immediately feeds 4 matmuls, and consecutive matmuls with
        # the same stationary operand sit adjacent in the PE stream.
        def proj_phase(groups, dst, lhs_of, rhs_of):
            for gb in range(0, len(groups), 4):
                batch = groups[gb:gb + 4]
                accs = [ps.tile([P, RC], F32, tag="mm", name="acc_p")
                        for _ in batch]
                for i in range(N_IB):
                    for a, g in zip(accs, batch):
                        nc.tensor.matmul(a, lhsT=lhs_of(i, g),
                                         rhs=rhs_of(i, g),
                                         start=(i == 0), stop=(i == N_IB - 1))
                for a, g in zip(accs, batch):
                    nc.vector.tensor_copy(dst(g), a)

        # K^T and Q^T packed for fp8 DoubleRow score matmuls: contraction
        # o = 256*ob + 128*e + p lives at tile ob, pack slot e, partition p.
        kt8 = [sb.tile([P, 2, NK], FP8, tag=f"kt8{ob}", name=f"kt8{ob}")
               for ob in range(N_IB // 2)]
        proj_phase([(o, jc) for o in range(N_IB) for jc in range(NK // RC)],
                   dst=lambda g: kt8[g[0] // 2][:, g[0] % 2, ts(g[1], RC)],
                   lhs_of=lambda i, g: w_sb["wk"][i][:, ts(g[0], P)],
                   rhs_of=lambda i, g: xkv_sb[i][:, ts(g[1], RC)])

        v_sb = [sb.tile([P, D], BF16, tag=f"vj{j}", name=f"vj{j}")
                for j in range(N_KT)]
        proj_phase([(j, dc) for j in range(N_KT) for dc in range(D // RC)],
                   dst=lambda g: v_sb[g[0]][:, ts(g[1], RC)],
                   lhs_of=lambda i, g: xkv_sb[i][:, ts(g[0], P)],
                   rhs_of=lambda i, g: w_sb["wv"][i][:, ts(g[1], RC)])

        qt8 = [sb.tile([P, 2, NQ], FP8, tag=f"qt8{ob}", name=f"qt8{ob}")
               for ob in range(N_IB // 2)]
        proj_phase([(o, rc) for o in range(N_IB) for rc in range(N_RC)],
                   dst=lambda g: qt8[g[0] // 2][:, g[0] % 2, ts(g[1], RC)],
                   lhs_of=lambda i, g: w_sb["wq"][i][:, ts(g[0], P)],
                   rhs_of=lambda i, g: xt_sb[i][:, ts(g[1], RC)])

        # ---- attention ----
        pt_tiles = {}

        def emit_st(rc):
            kts = _kept_kts(rc)
            lp = psl.tile([1, RC], F32, tag="lp", name="lp")
            for n, kt in enumerate(kts):
                acc_st = ps.tile([P, RC], F32, tag="mm", name="acc_st")
                nob = N_IB // 2
                for ob in range(nob):
                    nc.tensor.matmul(acc_st,
                                     lhsT=kt8[ob][:, :, ts(kt, P)],
                                     rhs=qt8[ob][:, :, ts(rc, RC)],
                                     start=(ob == 0), stop=(ob == nob - 1),
                                     perf_mode=mybir.MatmulPerfMode.DoubleRow)
                pt = pts.tile([P, RC], BF16, tag=f"pt{kt}", name=f"pt{kt}")
                nc.scalar.activation(pt, acc_st,
                                     mybir.ActivationFunctionType.Exp,
                                     scale=SCALE)
                base = _mask_base(rc, kt)
                if base < P - 1:  # tile straddles the causal diagonal
                    nc.gpsimd.affine_select(
                        out=pt, in_=pt,
                        compare_op=mybir.AluOpType.is_ge, fill=0.0,
                        base=base, channel_multiplier=-1, pattern=[[1, RC]])
                nc.tensor.matmul(lp, lhsT=ones, rhs=pt,
                                 start=(n == 0), stop=(n == len(kts) - 1))
                pt_tiles[(kt, rc)] = pt
            return lp

        def emit_pv(rc, lp):
            kts = _kept_kts(rc)
            for db in range(N_IB):
                acc_pv = ps.tile([P, RC], F32, tag="mm", name="acc_pv")
                for n, kt in enumerate(kts):
                    nc.tensor.matmul(acc_pv,
                                     lhsT=v_sb[kt][:, ts(db, P)],
                                     rhs=pt_tiles[(kt, rc)],
                                     start=(n == 0), stop=(n == len(kts) - 1))
                o_sb = outp.tile([P, RC], F32, tag="osb", name="osb")
                nc.vector.tensor_copy(o_sb, acc_pv)
                nc.sync.dma_start(out=ot[ts(db, P), ts(rc, RC)], in_=o_sb)
            l_sb = outp.tile([1, RC], F32, tag="lsb", name="lsb")
            nc.vector.tensor_copy(l_sb, lp)
            nc.sync.dma_start(out=ls[0:1, ts(rc, RC)], in_=l_sb)

        # software-pipelined emission: ST(rc+1) overlaps exp/mask of rc
        lp0 = emit_st(0)
        lp1 = emit_st(1)
        emit_pv(0, lp0)
        lp2 = emit_st(2)
        emit_pv(1, lp1)
        lp3 = emit_st(3)
        emit_pv(2, lp2)
        emit_pv(3, lp3)


_NC_CACHE = {}


def _get_nc():
    if "nc" not in _NC_CACHE:
        nc = bacc.Bacc("TRN2", target_bir_lowering=False, debug=False,
                       enable_asserts=False, num_devices=N_CORES)
        xt = nc.dram_tensor("xt", [D, NQ], BF16, kind="ExternalInput").ap()
        wqt = nc.dram_tensor("wqt", [D, D], BF16, kind="ExternalInput").ap()
        wkt = nc.dram_tensor("wkt", [D, D], BF16, kind="ExternalInput").ap()
        wvt = nc.dram_tensor("wvt", [D, D], BF16, kind="ExternalInput").ap()
        ot = nc.dram_tensor("ot", [D, NQ], F32, kind="ExternalOutput").ap()
        ls = nc.dram_tensor("ls", [1, NQ], F32, kind="ExternalOutput").ap()
        with tile.TileContext(nc) as tc:
            _emit(nc, tc, xt, wqt, wkt, wvt, ot, ls)
        nc.compile()
        _NC_CACHE["nc"] = nc
    return _NC_CACHE["nc"]


def make_in_maps(x, w_query, w_key, w_value):
    bf = ml_dtypes.bfloat16
    wqt = np.ascontiguousarray(np.asarray(w_query).T).astype(bf)
    wkt = np.ascontiguousarray(np.asarray(w_key).T).astype(bf)
    wvt = np.ascontiguousarray(np.asarray(w_value).T).astype(bf)
    in_maps = []
    for c in range(N_CORES):
        b, eta = c // 2, c % 2
        rows = (np.arange(NQ) + eta * STRIPE) % S  # cols past S wrap to junk
        xt_np = np.ascontiguousarray(np.asarray(x)[b, rows].T).astype(bf)
        in_maps.append({"xt": xt_np, "wqt": wqt, "wkt": wkt, "wvt": wvt})
    return in_maps


def merge_outputs(results):
    num = np.zeros((B, S, D), np.float32)
    den = np.zeros((B, S), np.float32)
    for c in range(N_CORES):
        b, eta = c // 2, c % 2
        otc = np.asarray(results[c]["ot"])   # [D, NQ]
        lc = np.asarray(results[c]["ls"])[0]  # [NQ]
        beta = eta * STRIPE
        nvalid = S - beta
        num[b, beta:] += otc.T[:nvalid]
        den[b, beta:] += lc[:nvalid]
    return (num / den[:, :, None]).astype(np.float32)


def kernel(x, w_query, w_key, w_value, _trace=False):
    nc = _get_nc()
    in_maps = make_in_maps(x, w_query, w_key, w_value)
    res = bass_utils.run_bass_kernel_spmd(
        nc, in_maps, core_ids=list(range(N_CORES)), trace=_trace)
    out = merge_outputs(res.results)
    if _trace:
        kernel.last_result = res
    return out
